# revision 11
# baseline (speedup 1.0000x reference)
"""CrossLayerTranscoder Trainium2 kernel.

Shards the d_transcoder (feature) axis across 8 NeuronCores (768 features
per layer per core).  Each core encodes its feature shard for all 6 layers
(acts kept feature-major on-chip), then decodes partial reconstructions for
every layer j accumulating over source layers i <= j.  The feature-shard
all-reduce is done on the host when unsharding (partials summed + b_dec).

Traffic optimizations over the bf16 baseline:
- W_dec is stored int8 with per-(source-layer, feature) scales folded into
  W_enc/b_enc on the host (s*relu(z) == relu(s*z) for s>0), so the device
  only needs an int8->bf16 CAST, no multiply.  Halves the dominant HBM
  stream (24.8 MB -> 12.4 MB per core).
- The cast for the 21 decoder pair-tiles is split across three paths to
  balance HBM read bw, SBUF write fabric, and engine throughput:
  'S' = gpsimd SWDGE casting DMA (DRAM int8 -> SBUF bf16, no engine cost),
  'V' = HWDGE int8 DMA + DVE tensor_copy, 'C' = same + ScalarE copy.
- Encode layer l and decode block j=l are interleaved so decode matmuls
  start early (warms the PE HAM clock gate) and the tensor queue is not
  serialized behind the whole encode DMA stream.
- Output partials are written bf16 (summed in f32 on the host).
"""

import numpy as np

import concourse.bass as bass
import concourse.mybir as mybir
from concourse.bass import ts
from concourse.tile import TileContext
from concourse.bass_utils import run_bass_kernel_spmd

L = 6            # layers
T = 128          # tokens
D = 768          # d_model
DT = 6144        # d_transcoder
N_CORES = 8
F = DT // N_CORES   # features per layer per core = 768
KD = D // 128       # d_model chunks of 128 = 6
KF = F // 128       # feature chunks of 128 = 6
# decode pairs in j-outer order (only upper triangle j >= i is nonzero)
PAIRS = [(i, j) for j in range(L) for i in range(j + 1)]
PAIR_IDX = {p: n for n, p in enumerate(PAIRS)}
NP = len(PAIRS)

F32 = mybir.dt.float32
BF16 = mybir.dt.bfloat16
I8 = mybir.dt.int8

# dequant path per pair (j-outer order):
#   'S' = gpsimd cast DMA (DRAM int8 -> SBUF bf16, SWDGE datapath) — the
#         steady-state stream, ~2.8us/pair, runs the whole kernel.
#   'E' = HWDGE int8 DMA prefetched early + dequant split across DVE
#         (kf 0-2) and ScalarE (kf 3-5) while those engines are idle.
# The E pairs are the late dec4/dec5 tiles: pre-dequanting them shortens
# the SWDGE chain to 15 pairs (~43us) and removes the tail stall.
E_PAIRS = [13, 14, 15, 16, 17, 18]
PATH = ['E' if n in E_PAIRS else 'S' for n in range(NP)]
N_E = len(E_PAIRS)


def _split_multiwaits(nc):
    """This container's walrus rejects >1 sync-wait per instruction; split
    extra waits onto same-engine NOPs inserted immediately before."""
    for fn in nc.m.functions:
        for bb in fn.blocks:
            new = []
            for ins in bb.instructions:
                si = ins.sync_info
                if si is not None and si.on_wait and len(si.on_wait) > 1:
                    waits = list(si.on_wait)
                    for w in waits[:-1]:
                        nop = mybir.InstNoOp(
                            name=nc.get_next_instruction_name(),
                            engine=ins.engine,
                            ins=[],
                            outs=[],
                            sync_info=mybir.SyncInfo(on_wait=[w], on_update=[]),
                        )
                        new.append(nop)
                    ins.sync_info = mybir.SyncInfo(
                        on_wait=[waits[-1]], on_update=list(si.on_update or [])
                    )
                new.append(ins)
            bb.instructions = new


def _build_nc():
    nc = bass.Bass()
    xt_d = nc.dram_tensor("xt", [L, 128, KD, T], BF16, kind="ExternalInput")
    we_d = nc.dram_tensor("we", [L, KD, 128, F], BF16, kind="ExternalInput")
    wq_d = nc.dram_tensor("wq", [NP, 128, KF, D], I8, kind="ExternalInput")
    be_d = nc.dram_tensor("be", [128, L, KF], F32, kind="ExternalInput")
    out_d = nc.dram_tensor("out", [L, 128, D], BF16, kind="ExternalOutput")

    with TileContext(nc) as tc:
        with (
            tc.tile_pool(name="const", bufs=1) as cpool,
            tc.tile_pool(name="w", bufs=3) as wpool,
            tc.tile_pool(name="s", bufs=7) as spool,
            tc.tile_pool(name="e", bufs=N_E) as epool,
            tc.tile_pool(name="q", bufs=4) as qpool,
            tc.tile_pool(name="pse", bufs=2, space="PSUM") as pse,
            tc.tile_pool(name="psd", bufs=4, space="PSUM") as psd,
        ):
            X = cpool.tile([128, L, KD, T], BF16, tag="x")
            BE = cpool.tile([128, L, KF], F32, tag="be")
            A = cpool.tile([128, L, KF, T], BF16, tag="acts")
            OUT = cpool.tile([128, L, D], BF16, tag="out")

            # ---- PE warm-up: ~32 dummy matmuls during the framework
            # preamble flip the HAM clock gate to 8/8 (2.4 GHz) before the
            # first real matmul; the PE is otherwise idle here.
            WRM = cpool.tile([128, 128], BF16, tag="wrm")
            nc.vector.memset(WRM[:], 0.0)
            wps = pse.tile([128, 128], F32, tag="pse")
            for _ in range(32):
                nc.tensor.matmul(wps[:], WRM[:], WRM[:], start=True, stop=True)

            nc.sync.dma_start(out=BE[:], in_=be_d[:])

            e_wd = {}   # pair idx -> pre-dequanted bf16 tile
            e_wq = {}   # pair idx -> int8 landing tile
            for l in range(L):
                # ---- encode layer l: acts[f, t] = relu(We^T-chunks @ x^T + b)
                nc.sync.dma_start(out=X[:, l, :, :], in_=xt_d[l])
                we = wpool.tile([128, KD, F], BF16, tag="w")
                for kd in range(KD):
                    nc.sync.dma_start(out=we[:, kd, :], in_=we_d[l, kd])
                # prefetch two E-pair int8 tiles per early layer (l=1..3)
                if 1 <= l <= 3:
                    for n in E_PAIRS[2 * (l - 1) : 2 * l]:
                        wq = qpool.tile([128, KF, D], I8, tag="q")
                        nc.sync.dma_start(out=wq[:], in_=wq_d[n])
                        e_wq[n] = wq
                for ft in range(KF):
                    ps = pse.tile([128, T], F32, tag="pse")
                    for kd in range(KD):
                        nc.tensor.matmul(
                            ps[:],
                            we[:, kd, ts(ft, 128)],
                            X[:, l, kd, :],
                            start=(kd == 0),
                            stop=(kd == KD - 1),
                        )
                    nc.vector.tensor_scalar(
                        out=A[:, l, ft, :],
                        in0=ps[:],
                        scalar1=BE[:, l, ts(ft, 1)],
                        scalar2=0.0,
                        op0=mybir.AluOpType.add,
                        op1=mybir.AluOpType.max,
                    )
                # dequant the prefetched E pairs while DVE/ScalarE are idle
                if 2 <= l <= 4:
                    for n in E_PAIRS[2 * (l - 2) : 2 * (l - 1)]:
                        wd = epool.tile([128, KF, D], BF16, tag="e")
                        wq = e_wq.pop(n)
                        nc.vector.tensor_copy(out=wd[:, 0:3, :], in_=wq[:, 0:3, :])
                        nc.scalar.copy(out=wd[:, 3:6, :], in_=wq[:, 3:6, :])
                        e_wd[n] = wd

                # ---- decode block j=l: recon_j += acts_i^T @ Wq[i,j] (i<=j)
                j = l
                ps0 = psd.tile([128, 384], F32, tag="psd")
                ps1 = psd.tile([128, 384], F32, tag="psd")
                for i in range(j + 1):
                    n = PAIR_IDX[(i, j)]
                    if PATH[n] == "S":
                        wd = spool.tile([128, KF, D], BF16, tag="s")
                        nc.gpsimd.dma_start(out=wd[:], in_=wq_d[n])
                    else:
                        wd = e_wd[n]
                    for kf in range(KF):
                        nc.tensor.matmul(
                            ps0[:], A[:, i, kf, :], wd[:, kf, 0:384],
                            start=(i == 0 and kf == 0),
                            stop=(i == j and kf == KF - 1),
                        )
                    for kf in range(KF):
                        nc.tensor.matmul(
                            ps1[:], A[:, i, kf, :], wd[:, kf, 384:768],
                            start=(i == 0 and kf == 0),
                            stop=(i == j and kf == KF - 1),
                        )
                nc.scalar.copy(out=OUT[:, j, 0:384], in_=ps0[:])
                nc.sync.dma_start(out=out_d[j, :, 0:384], in_=OUT[:, j, 0:384])
                nc.vector.tensor_copy(out=OUT[:, j, 384:768], in_=ps1[:])
                nc.sync.dma_start(out=out_d[j, :, 384:768], in_=OUT[:, j, 384:768])

    _split_multiwaits(nc)
    return nc


_NC_CACHE = {}


def _get_nc():
    if "nc" not in _NC_CACHE:
        _NC_CACHE["nc"] = _build_nc()
    return _NC_CACHE["nc"]


def _bf16():
    import ml_dtypes

    return np.dtype(ml_dtypes.bfloat16)


def _prepare(x, W_enc, b_enc, W_dec, dec_mask):
    """Host-side quantization + per-core pre-swizzle into DMA layouts."""
    bf16 = _bf16()
    # per-(source-layer, feature) int8 scale over valid (j >= i) decoders
    if dec_mask is None:
        dec_mask = np.triu(np.ones((L, L), dtype=bool))
    Wd_m = np.where(dec_mask[:, :, None, None], W_dec, 0.0)
    s = np.abs(Wd_m).max(axis=(1, 3)) / 127.0  # [L, DT]
    s = np.where(s == 0, 1.0, s).astype(np.float32)

    # fold the scale into the encoder (relu(s*z) == s*relu(z), s > 0)
    W_enc_f = W_enc * s[:, :, None]
    b_enc_f = b_enc * s

    # xt[l, p, kd, t] = x[l, t, kd*128+p] — same on every core
    xt = np.ascontiguousarray(
        x.transpose(2, 0, 1).reshape(KD, 128, L, T).transpose(2, 1, 0, 3)
    ).astype(bf16)

    in_maps = []
    for c in range(N_CORES):
        fs = c * F
        w = W_enc_f[:, fs : fs + F, :]  # [L, F, D]
        we = np.ascontiguousarray(
            w.transpose(0, 2, 1).reshape(L, KD, 128, F)
        ).astype(bf16)
        be = np.ascontiguousarray(
            b_enc_f[:, fs : fs + F].reshape(L, KF, 128).transpose(2, 0, 1)
        ).astype(np.float32)
        in_maps.append({"xt": xt, "we": we, "be": be})

    # quantize + pack decoder shards: wq[pair, p, kf, d]
    for c in range(N_CORES):
        fs = c * F
        wq = np.empty((NP, 128, KF, D), dtype=np.int8)
        for n, (i, j) in enumerate(PAIRS):
            blk = W_dec[i, j, fs : fs + F, :] / s[i, fs : fs + F, None]
            q = np.rint(blk).clip(-127, 127).astype(np.int8)  # [F, D]
            wq[n] = q.reshape(KF, 128, D).transpose(1, 0, 2)
        in_maps[c]["wq"] = wq
    return in_maps


def kernel(x, W_enc, b_enc, b_dec, W_dec, dec_mask=None, **_unused):
    x = np.asarray(x, dtype=np.float32)
    W_enc = np.asarray(W_enc, dtype=np.float32)
    b_enc = np.asarray(b_enc, dtype=np.float32)
    b_dec = np.asarray(b_dec, dtype=np.float32)
    W_dec = np.asarray(W_dec, dtype=np.float32)

    nc = _get_nc()
    in_maps = _prepare(x, W_enc, b_enc, W_dec, dec_mask)
    res = run_bass_kernel_spmd(nc, in_maps, core_ids=list(range(N_CORES)))

    # host-side all-reduce over feature shards + decoder bias
    recon = np.zeros((L, T, D), dtype=np.float32)
    for c in range(N_CORES):
        recon += res.results[c]["out"].astype(np.float32)
    recon += b_dec[:, None, :]
    return recon


# revision 14
# speedup vs baseline: 1.0616x; 1.0616x over previous
"""CrossLayerTranscoder Trainium2 kernel.

Shards the d_transcoder (feature) axis across 8 NeuronCores (768 features
per layer per core).  Each core encodes its feature shard for all 6 layers
(acts kept feature-major on-chip), then decodes partial reconstructions for
every layer j accumulating over source layers i <= j.  The feature-shard
all-reduce is done on the host when unsharding (partials summed + b_dec).

Traffic optimizations over the bf16 baseline:
- W_dec is stored int8 with per-(source-layer, feature) scales folded into
  W_enc/b_enc on the host (s*relu(z) == relu(s*z) for s>0), so the device
  only needs an int8->bf16 CAST, no multiply.  Halves the dominant HBM
  stream (24.8 MB -> 12.4 MB per core).
- The cast for the 21 decoder pair-tiles is split across three paths to
  balance HBM read bw, SBUF write fabric, and engine throughput:
  'S' = gpsimd SWDGE casting DMA (DRAM int8 -> SBUF bf16, no engine cost),
  'V' = HWDGE int8 DMA + DVE tensor_copy, 'C' = same + ScalarE copy.
- Encode layer l and decode block j=l are interleaved so decode matmuls
  start early (warms the PE HAM clock gate) and the tensor queue is not
  serialized behind the whole encode DMA stream.
- Output partials are written bf16 (summed in f32 on the host).
"""

import numpy as np

import concourse.bass as bass
import concourse.mybir as mybir
from concourse.bass import ts
from concourse.tile import TileContext
from concourse.bass_utils import run_bass_kernel_spmd

L = 6            # layers
T = 128          # tokens
D = 768          # d_model
DT = 6144        # d_transcoder
N_CORES = 8
F = DT // N_CORES   # features per layer per core = 768
KD = D // 128       # d_model chunks of 128 = 6
KF = F // 128       # feature chunks of 128 = 6
# decode pairs in j-outer order (only upper triangle j >= i is nonzero)
PAIRS = [(i, j) for j in range(L) for i in range(j + 1)]
PAIR_IDX = {p: n for n, p in enumerate(PAIRS)}
NP = len(PAIRS)

F32 = mybir.dt.float32
BF16 = mybir.dt.bfloat16
I8 = mybir.dt.int8

# dequant path per pair (j-outer order):
#   'S' = gpsimd cast DMA (DRAM int8 -> SBUF bf16, SWDGE datapath) — the
#         steady-state stream, ~2.8us/pair, runs the whole kernel.
#   'E' = HWDGE int8 DMA prefetched early + dequant split across DVE
#         (kf 0-2) and ScalarE (kf 3-5) while those engines are idle.
# counts: S=8, E=13 balances SWDGE queue vs engine throughput under the
# HBM read roofline.
PATH = ['E' if (n % 3) or n == 20 else 'S' for n in range(NP)]
PATH[20] = 'S'
N_E = PATH.count('E')


def _split_multiwaits(nc):
    """This container's walrus rejects >1 sync-wait per instruction; split
    extra waits onto same-engine NOPs inserted immediately before."""
    for fn in nc.m.functions:
        for bb in fn.blocks:
            new = []
            for ins in bb.instructions:
                si = ins.sync_info
                if si is not None and si.on_wait and len(si.on_wait) > 1:
                    waits = list(si.on_wait)
                    for w in waits[:-1]:
                        nop = mybir.InstNoOp(
                            name=nc.get_next_instruction_name(),
                            engine=ins.engine,
                            ins=[],
                            outs=[],
                            sync_info=mybir.SyncInfo(on_wait=[w], on_update=[]),
                        )
                        new.append(nop)
                    ins.sync_info = mybir.SyncInfo(
                        on_wait=[waits[-1]], on_update=list(si.on_update or [])
                    )
                new.append(ins)
            bb.instructions = new


def _build_nc():
    nc = bass.Bass()
    xt_d = nc.dram_tensor("xt", [L, 128, KD, T], BF16, kind="ExternalInput")
    we_d = nc.dram_tensor("we", [L, KD, 128, F], BF16, kind="ExternalInput")
    wq_d = nc.dram_tensor("wq", [NP, 128, KF, D], I8, kind="ExternalInput")
    be_d = nc.dram_tensor("be", [128, L, KF], F32, kind="ExternalInput")
    out_d = nc.dram_tensor("out", [L, 128, D], BF16, kind="ExternalOutput")

    with TileContext(nc) as tc:
        with (
            tc.tile_pool(name="const", bufs=1) as cpool,
            tc.tile_pool(name="w", bufs=9) as wpool,
            tc.tile_pool(name="q", bufs=N_E) as qpool,
            tc.tile_pool(name="pse", bufs=2, space="PSUM") as pse,
            tc.tile_pool(name="psd", bufs=4, space="PSUM") as psd,
        ):
            X = cpool.tile([128, L, KD, T], BF16, tag="x")
            BE = cpool.tile([128, L, KF], F32, tag="be")
            A = cpool.tile([128, L, KF, T], BF16, tag="acts")
            OUT = cpool.tile([128, L, D], BF16, tag="out")

            # ---- PE warm-up: ~32 dummy matmuls during the framework
            # preamble flip the HAM clock gate to 8/8 (2.4 GHz) before the
            # first real matmul; the PE is otherwise idle here.
            WRM = cpool.tile([128, 128], BF16, tag="wrm")
            nc.vector.memset(WRM[:], 0.0)
            wps = pse.tile([128, 128], F32, tag="pse")
            for _ in range(32):
                nc.tensor.matmul(wps[:], WRM[:], WRM[:], start=True, stop=True)

            nc.sync.dma_start(out=BE[:], in_=be_d[:])

            for l in range(L):
                # ---- encode layer l: acts[f, t] = relu(We^T-chunks @ x^T + b)
                nc.sync.dma_start(out=X[:, l, :, :], in_=xt_d[l])
                we = wpool.tile([128, KD, F], BF16, tag="w")
                for kd in range(KD):
                    nc.sync.dma_start(out=we[:, kd, :], in_=we_d[l, kd])
                for ft in range(KF):
                    ps = pse.tile([128, T], F32, tag="pse")
                    for kd in range(KD):
                        nc.tensor.matmul(
                            ps[:],
                            we[:, kd, ts(ft, 128)],
                            X[:, l, kd, :],
                            start=(kd == 0),
                            stop=(kd == KD - 1),
                        )
                    nc.vector.tensor_scalar(
                        out=A[:, l, ft, :],
                        in0=ps[:],
                        scalar1=BE[:, l, ts(ft, 1)],
                        scalar2=0.0,
                        op0=mybir.AluOpType.add,
                        op1=mybir.AluOpType.max,
                    )

                # ---- decode block j=l: recon_j += acts_i^T @ Wq[i,j] (i<=j)
                # Issue all loads first; consume E pairs (bounded dequant
                # latency) before S pairs (SWDGE queue latency).  PSUM
                # accumulation order over i is free — only the start/stop
                # flags must follow the executed order.
                j = l
                order = [i for i in range(j + 1) if PATH[PAIR_IDX[(i, j)]] == "E"]
                order += [i for i in range(j + 1) if PATH[PAIR_IDX[(i, j)]] == "S"]
                tiles = {}
                for i in order:
                    n = PAIR_IDX[(i, j)]
                    wd = wpool.tile([128, KF, D], BF16, tag="w")
                    if PATH[n] == "S":
                        nc.gpsimd.dma_start(out=wd[:], in_=wq_d[n])
                    else:
                        wq = qpool.tile([128, KF, D], I8, tag="q")
                        nc.sync.dma_start(out=wq[:], in_=wq_d[n])
                        nc.vector.tensor_copy(out=wd[:, 0:3, :], in_=wq[:, 0:3, :])
                        nc.scalar.copy(out=wd[:, 3:6, :], in_=wq[:, 3:6, :])
                    tiles[i] = wd
                ps0 = psd.tile([128, 384], F32, tag="psd")
                ps1 = psd.tile([128, 384], F32, tag="psd")
                for k, i in enumerate(order):
                    wd = tiles[i]
                    for kf in range(KF):
                        nc.tensor.matmul(
                            ps0[:], A[:, i, kf, :], wd[:, kf, 0:384],
                            start=(k == 0 and kf == 0),
                            stop=(k == j and kf == KF - 1),
                        )
                    for kf in range(KF):
                        nc.tensor.matmul(
                            ps1[:], A[:, i, kf, :], wd[:, kf, 384:768],
                            start=(k == 0 and kf == 0),
                            stop=(k == j and kf == KF - 1),
                        )
                nc.scalar.copy(out=OUT[:, j, 0:384], in_=ps0[:])
                nc.sync.dma_start(out=out_d[j, :, 0:384], in_=OUT[:, j, 0:384])
                nc.vector.tensor_copy(out=OUT[:, j, 384:768], in_=ps1[:])
                nc.sync.dma_start(out=out_d[j, :, 384:768], in_=OUT[:, j, 384:768])

    _split_multiwaits(nc)
    return nc


_NC_CACHE = {}


def _get_nc():
    if "nc" not in _NC_CACHE:
        _NC_CACHE["nc"] = _build_nc()
    return _NC_CACHE["nc"]


def _bf16():
    import ml_dtypes

    return np.dtype(ml_dtypes.bfloat16)


def _prepare(x, W_enc, b_enc, W_dec, dec_mask):
    """Host-side quantization + per-core pre-swizzle into DMA layouts."""
    bf16 = _bf16()
    # per-(source-layer, feature) int8 scale over valid (j >= i) decoders
    if dec_mask is None:
        dec_mask = np.triu(np.ones((L, L), dtype=bool))
    Wd_m = np.where(dec_mask[:, :, None, None], W_dec, 0.0)
    s = np.abs(Wd_m).max(axis=(1, 3)) / 127.0  # [L, DT]
    s = np.where(s == 0, 1.0, s).astype(np.float32)

    # fold the scale into the encoder (relu(s*z) == s*relu(z), s > 0)
    W_enc_f = W_enc * s[:, :, None]
    b_enc_f = b_enc * s

    # xt[l, p, kd, t] = x[l, t, kd*128+p] — same on every core
    xt = np.ascontiguousarray(
        x.transpose(2, 0, 1).reshape(KD, 128, L, T).transpose(2, 1, 0, 3)
    ).astype(bf16)

    in_maps = []
    for c in range(N_CORES):
        fs = c * F
        w = W_enc_f[:, fs : fs + F, :]  # [L, F, D]
        we = np.ascontiguousarray(
            w.transpose(0, 2, 1).reshape(L, KD, 128, F)
        ).astype(bf16)
        be = np.ascontiguousarray(
            b_enc_f[:, fs : fs + F].reshape(L, KF, 128).transpose(2, 0, 1)
        ).astype(np.float32)
        in_maps.append({"xt": xt, "we": we, "be": be})

    # quantize + pack decoder shards: wq[pair, p, kf, d]
    for c in range(N_CORES):
        fs = c * F
        wq = np.empty((NP, 128, KF, D), dtype=np.int8)
        for n, (i, j) in enumerate(PAIRS):
            blk = W_dec[i, j, fs : fs + F, :] / s[i, fs : fs + F, None]
            q = np.rint(blk).clip(-127, 127).astype(np.int8)  # [F, D]
            wq[n] = q.reshape(KF, 128, D).transpose(1, 0, 2)
        in_maps[c]["wq"] = wq
    return in_maps


def kernel(x, W_enc, b_enc, b_dec, W_dec, dec_mask=None, **_unused):
    x = np.asarray(x, dtype=np.float32)
    W_enc = np.asarray(W_enc, dtype=np.float32)
    b_enc = np.asarray(b_enc, dtype=np.float32)
    b_dec = np.asarray(b_dec, dtype=np.float32)
    W_dec = np.asarray(W_dec, dtype=np.float32)

    nc = _get_nc()
    in_maps = _prepare(x, W_enc, b_enc, W_dec, dec_mask)
    res = run_bass_kernel_spmd(nc, in_maps, core_ids=list(range(N_CORES)))

    # host-side all-reduce over feature shards + decoder bias
    recon = np.zeros((L, T, D), dtype=np.float32)
    for c in range(N_CORES):
        recon += res.results[c]["out"].astype(np.float32)
    recon += b_dec[:, None, :]
    return recon


# revision 15
# speedup vs baseline: 1.1664x; 1.0987x over previous
"""CrossLayerTranscoder Trainium2 kernel.

Shards the d_transcoder (feature) axis across 8 NeuronCores (768 features
per layer per core).  Each core encodes its feature shard for all 6 layers
(acts kept feature-major on-chip), then decodes partial reconstructions for
every layer j accumulating over source layers i <= j.  The feature-shard
all-reduce is done on the host when unsharding (partials summed + b_dec).

Traffic optimizations over the bf16 baseline:
- W_dec is stored int8 with per-(source-layer, feature) scales folded into
  W_enc/b_enc on the host (s*relu(z) == relu(s*z) for s>0), so the device
  only needs an int8->bf16 CAST, no multiply.  Halves the dominant HBM
  stream (24.8 MB -> 12.4 MB per core).
- The cast for the 21 decoder pair-tiles is split across three paths to
  balance HBM read bw, SBUF write fabric, and engine throughput:
  'S' = gpsimd SWDGE casting DMA (DRAM int8 -> SBUF bf16, no engine cost),
  'V' = HWDGE int8 DMA + DVE tensor_copy, 'C' = same + ScalarE copy.
- Encode layer l and decode block j=l are interleaved so decode matmuls
  start early (warms the PE HAM clock gate) and the tensor queue is not
  serialized behind the whole encode DMA stream.
- Output partials are written bf16 (summed in f32 on the host).
"""

import numpy as np

import concourse.bass as bass
import concourse.mybir as mybir
from concourse.bass import ts
from concourse.tile import TileContext
from concourse.bass_utils import run_bass_kernel_spmd

L = 6            # layers
T = 128          # tokens
D = 768          # d_model
DT = 6144        # d_transcoder
N_CORES = 8
F = DT // N_CORES   # features per layer per core = 768
KD = D // 128       # d_model chunks of 128 = 6
KF = F // 128       # feature chunks of 128 = 6
# decode pairs in j-outer order (only upper triangle j >= i is nonzero)
PAIRS = [(i, j) for j in range(L) for i in range(j + 1)]
PAIR_IDX = {p: n for n, p in enumerate(PAIRS)}
NP = len(PAIRS)

F32 = mybir.dt.float32
BF16 = mybir.dt.bfloat16
I8 = mybir.dt.int8

# dequant path per pair (j-outer order):
#   'S' = gpsimd cast DMA (DRAM int8 -> SBUF bf16, SWDGE datapath) — the
#         steady-state stream, ~2.8us/pair, runs the whole kernel.
#   'E' = HWDGE int8 DMA prefetched early + dequant split across DVE
#         (kf 0-2) and ScalarE (kf 3-5) while those engines are idle.
# counts: S=8, E=13 balances SWDGE queue vs engine throughput under the
# HBM read roofline.
PATH = ['E' if (n % 3) or n == 20 else 'S' for n in range(NP)]
PATH[20] = 'S'
N_E = PATH.count('E')


def _split_multiwaits(nc):
    """This container's walrus rejects >1 sync-wait per instruction; split
    extra waits onto same-engine NOPs inserted immediately before."""
    for fn in nc.m.functions:
        for bb in fn.blocks:
            new = []
            for ins in bb.instructions:
                si = ins.sync_info
                if si is not None and si.on_wait and len(si.on_wait) > 1:
                    waits = list(si.on_wait)
                    for w in waits[:-1]:
                        nop = mybir.InstNoOp(
                            name=nc.get_next_instruction_name(),
                            engine=ins.engine,
                            ins=[],
                            outs=[],
                            sync_info=mybir.SyncInfo(on_wait=[w], on_update=[]),
                        )
                        new.append(nop)
                    ins.sync_info = mybir.SyncInfo(
                        on_wait=[waits[-1]], on_update=list(si.on_update or [])
                    )
                new.append(ins)
            bb.instructions = new


def _build_nc():
    nc = bass.Bass()
    xt_d = nc.dram_tensor("xt", [L, 128, KD, T], BF16, kind="ExternalInput")
    we_d = nc.dram_tensor("we", [L, KD, 128, F], BF16, kind="ExternalInput")
    wq_d = nc.dram_tensor("wq", [NP, 128, KF, D], I8, kind="ExternalInput")
    be_d = nc.dram_tensor("be", [128, L, KF], F32, kind="ExternalInput")
    out_d = nc.dram_tensor("out", [L, 128, D], BF16, kind="ExternalOutput")

    with TileContext(nc) as tc:
        with (
            tc.tile_pool(name="const", bufs=1) as cpool,
            tc.tile_pool(name="w", bufs=9) as wpool,
            tc.tile_pool(name="q", bufs=N_E) as qpool,
            tc.tile_pool(name="pse", bufs=2, space="PSUM") as pse,
            tc.tile_pool(name="psd", bufs=4, space="PSUM") as psd,
        ):
            X = cpool.tile([128, L, KD, T], BF16, tag="x")
            BE = cpool.tile([128, L, KF], F32, tag="be")
            A = cpool.tile([128, L, KF, T], BF16, tag="acts")
            OUT = cpool.tile([128, L, D], BF16, tag="out")

            # ---- PE warm-up: ~32 dummy matmuls during the framework
            # preamble flip the HAM clock gate to 8/8 (2.4 GHz) before the
            # first real matmul; the PE is otherwise idle here.
            WRM = cpool.tile([128, 128], BF16, tag="wrm")
            nc.vector.memset(WRM[:], 0.0)
            wps = pse.tile([128, 128], F32, tag="pse")
            for _ in range(32):
                nc.tensor.matmul(wps[:], WRM[:], WRM[:], start=True, stop=True)

            nc.sync.dma_start(out=BE[:], in_=be_d[:])

            for l in range(L):
                # ---- encode layer l: acts[f, t] = relu(We^T-chunks @ x^T + b)
                nc.sync.dma_start(out=X[:, l, :, :], in_=xt_d[l])
                we = wpool.tile([128, KD, F], BF16, tag="w")
                for kd in range(KD):
                    nc.sync.dma_start(out=we[:, kd, :], in_=we_d[l, kd])
                for ft in range(KF):
                    ps = pse.tile([128, T], F32, tag="pse")
                    for kd in range(KD):
                        nc.tensor.matmul(
                            ps[:],
                            we[:, kd, ts(ft, 128)],
                            X[:, l, kd, :],
                            start=(kd == 0),
                            stop=(kd == KD - 1),
                        )
                    nc.vector.tensor_scalar(
                        out=A[:, l, ft, :],
                        in0=ps[:],
                        scalar1=BE[:, l, ts(ft, 1)],
                        scalar2=0.0,
                        op0=mybir.AluOpType.add,
                        op1=mybir.AluOpType.max,
                    )

                # ---- decode block j=l: recon_j += acts_i^T @ Wq[i,j] (i<=j)
                j = l
                ps0 = psd.tile([128, 384], F32, tag="psd")
                ps1 = psd.tile([128, 384], F32, tag="psd")
                for i in range(j + 1):
                    n = PAIR_IDX[(i, j)]
                    wd = wpool.tile([128, KF, D], BF16, tag="w")
                    if PATH[n] == "S":
                        nc.gpsimd.dma_start(out=wd[:], in_=wq_d[n])
                    else:
                        wq = qpool.tile([128, KF, D], I8, tag="q")
                        nc.sync.dma_start(out=wq[:], in_=wq_d[n])
                        nc.vector.tensor_copy(out=wd[:, 0:3, :], in_=wq[:, 0:3, :])
                        nc.scalar.copy(out=wd[:, 3:6, :], in_=wq[:, 3:6, :])
                    for kf in range(KF):
                        nc.tensor.matmul(
                            ps0[:], A[:, i, kf, :], wd[:, kf, 0:384],
                            start=(i == 0 and kf == 0),
                            stop=(i == j and kf == KF - 1),
                        )
                    for kf in range(KF):
                        nc.tensor.matmul(
                            ps1[:], A[:, i, kf, :], wd[:, kf, 384:768],
                            start=(i == 0 and kf == 0),
                            stop=(i == j and kf == KF - 1),
                        )
                nc.scalar.copy(out=OUT[:, j, 0:384], in_=ps0[:])
                nc.sync.dma_start(out=out_d[j, :, 0:384], in_=OUT[:, j, 0:384])
                nc.vector.tensor_copy(out=OUT[:, j, 384:768], in_=ps1[:])
                nc.sync.dma_start(out=out_d[j, :, 384:768], in_=OUT[:, j, 384:768])

    _split_multiwaits(nc)
    return nc


_NC_CACHE = {}


def _get_nc():
    if "nc" not in _NC_CACHE:
        _NC_CACHE["nc"] = _build_nc()
    return _NC_CACHE["nc"]


def _bf16():
    import ml_dtypes

    return np.dtype(ml_dtypes.bfloat16)


def _prepare(x, W_enc, b_enc, W_dec, dec_mask):
    """Host-side quantization + per-core pre-swizzle into DMA layouts."""
    bf16 = _bf16()
    # per-(source-layer, feature) int8 scale over valid (j >= i) decoders
    if dec_mask is None:
        dec_mask = np.triu(np.ones((L, L), dtype=bool))
    Wd_m = np.where(dec_mask[:, :, None, None], W_dec, 0.0)
    s = np.abs(Wd_m).max(axis=(1, 3)) / 127.0  # [L, DT]
    s = np.where(s == 0, 1.0, s).astype(np.float32)

    # fold the scale into the encoder (relu(s*z) == s*relu(z), s > 0)
    W_enc_f = W_enc * s[:, :, None]
    b_enc_f = b_enc * s

    # xt[l, p, kd, t] = x[l, t, kd*128+p] — same on every core
    xt = np.ascontiguousarray(
        x.transpose(2, 0, 1).reshape(KD, 128, L, T).transpose(2, 1, 0, 3)
    ).astype(bf16)

    in_maps = []
    for c in range(N_CORES):
        fs = c * F
        w = W_enc_f[:, fs : fs + F, :]  # [L, F, D]
        we = np.ascontiguousarray(
            w.transpose(0, 2, 1).reshape(L, KD, 128, F)
        ).astype(bf16)
        be = np.ascontiguousarray(
            b_enc_f[:, fs : fs + F].reshape(L, KF, 128).transpose(2, 0, 1)
        ).astype(np.float32)
        in_maps.append({"xt": xt, "we": we, "be": be})

    # quantize + pack decoder shards: wq[pair, p, kf, d]
    for c in range(N_CORES):
        fs = c * F
        wq = np.empty((NP, 128, KF, D), dtype=np.int8)
        for n, (i, j) in enumerate(PAIRS):
            blk = W_dec[i, j, fs : fs + F, :] / s[i, fs : fs + F, None]
            q = np.rint(blk).clip(-127, 127).astype(np.int8)  # [F, D]
            wq[n] = q.reshape(KF, 128, D).transpose(1, 0, 2)
        in_maps[c]["wq"] = wq
    return in_maps


def kernel(x, W_enc, b_enc, b_dec, W_dec, dec_mask=None, **_unused):
    x = np.asarray(x, dtype=np.float32)
    W_enc = np.asarray(W_enc, dtype=np.float32)
    b_enc = np.asarray(b_enc, dtype=np.float32)
    b_dec = np.asarray(b_dec, dtype=np.float32)
    W_dec = np.asarray(W_dec, dtype=np.float32)

    nc = _get_nc()
    in_maps = _prepare(x, W_enc, b_enc, W_dec, dec_mask)
    res = run_bass_kernel_spmd(nc, in_maps, core_ids=list(range(N_CORES)))

    # host-side all-reduce over feature shards + decoder bias
    recon = np.zeros((L, T, D), dtype=np.float32)
    for c in range(N_CORES):
        recon += res.results[c]["out"].astype(np.float32)
    recon += b_dec[:, None, :]
    return recon


# revision 16
# speedup vs baseline: 1.1826x; 1.0139x over previous
"""CrossLayerTranscoder Trainium2 kernel.

Shards the d_transcoder (feature) axis across 8 NeuronCores (768 features
per layer per core).  Each core encodes its feature shard for all 6 layers
(acts kept feature-major on-chip), then decodes partial reconstructions for
every layer j accumulating over source layers i <= j.  The feature-shard
all-reduce is done on the host when unsharding (partials summed + b_dec).

Traffic optimizations over the bf16 baseline:
- W_dec is stored int8 with per-(source-layer, feature) scales folded into
  W_enc/b_enc on the host (s*relu(z) == relu(s*z) for s>0), so the device
  only needs an int8->bf16 CAST, no multiply.  Halves the dominant HBM
  stream (24.8 MB -> 12.4 MB per core).
- The cast for the 21 decoder pair-tiles is split across three paths to
  balance HBM read bw, SBUF write fabric, and engine throughput:
  'S' = gpsimd SWDGE casting DMA (DRAM int8 -> SBUF bf16, no engine cost),
  'V' = HWDGE int8 DMA + DVE tensor_copy, 'C' = same + ScalarE copy.
- Encode layer l and decode block j=l are interleaved so decode matmuls
  start early (warms the PE HAM clock gate) and the tensor queue is not
  serialized behind the whole encode DMA stream.
- Output partials are written bf16 (summed in f32 on the host).
"""

import numpy as np

import concourse.bass as bass
import concourse.mybir as mybir
from concourse.bass import ts
from concourse.tile import TileContext
from concourse.bass_utils import run_bass_kernel_spmd

L = 6            # layers
T = 128          # tokens
D = 768          # d_model
DT = 6144        # d_transcoder
N_CORES = 8
F = DT // N_CORES   # features per layer per core = 768
KD = D // 128       # d_model chunks of 128 = 6
KF = F // 128       # feature chunks of 128 = 6
# decode pairs in j-outer order (only upper triangle j >= i is nonzero)
PAIRS = [(i, j) for j in range(L) for i in range(j + 1)]
PAIR_IDX = {p: n for n, p in enumerate(PAIRS)}
NP = len(PAIRS)

F32 = mybir.dt.float32
BF16 = mybir.dt.bfloat16
I8 = mybir.dt.int8

# dequant path per pair (j-outer order):
#   'S' = gpsimd cast DMA (DRAM int8 -> SBUF bf16, SWDGE datapath) — the
#         steady-state stream, ~2.8us/pair, runs the whole kernel.
#   'E' = HWDGE int8 DMA prefetched early + dequant split across DVE
#         (kf 0-2) and ScalarE (kf 3-5) while those engines are idle.
# counts: S=8, E=13 balances SWDGE queue vs engine throughput under the
# HBM read roofline.
PATH = ['E' if (n % 3) or n == 20 else 'S' for n in range(NP)]
PATH[20] = 'S'
N_E = PATH.count('E')


def _split_multiwaits(nc):
    """This container's walrus rejects >1 sync-wait per instruction; split
    extra waits onto same-engine NOPs inserted immediately before."""
    for fn in nc.m.functions:
        for bb in fn.blocks:
            new = []
            for ins in bb.instructions:
                si = ins.sync_info
                if si is not None and si.on_wait and len(si.on_wait) > 1:
                    waits = list(si.on_wait)
                    for w in waits[:-1]:
                        nop = mybir.InstNoOp(
                            name=nc.get_next_instruction_name(),
                            engine=ins.engine,
                            ins=[],
                            outs=[],
                            sync_info=mybir.SyncInfo(on_wait=[w], on_update=[]),
                        )
                        new.append(nop)
                    ins.sync_info = mybir.SyncInfo(
                        on_wait=[waits[-1]], on_update=list(si.on_update or [])
                    )
                new.append(ins)
            bb.instructions = new


def _build_nc():
    nc = bass.Bass()
    xt_d = nc.dram_tensor("xt", [L, 128, KD, T], BF16, kind="ExternalInput")
    we_d = nc.dram_tensor("we", [L, KD, 128, F], BF16, kind="ExternalInput")
    wq_d = nc.dram_tensor("wq", [NP, 128, KF, D], I8, kind="ExternalInput")
    be_d = nc.dram_tensor("be", [128, L, KF], F32, kind="ExternalInput")
    out_d = nc.dram_tensor("out", [L, 128, D], BF16, kind="ExternalOutput")

    with TileContext(nc) as tc:
        with (
            tc.tile_pool(name="const", bufs=1) as cpool,
            tc.tile_pool(name="w", bufs=9) as wpool,
            tc.tile_pool(name="q", bufs=N_E) as qpool,
            tc.tile_pool(name="pse", bufs=2, space="PSUM") as pse,
            tc.tile_pool(name="psd", bufs=6, space="PSUM") as psd,
        ):
            X = cpool.tile([128, L, KD, T], BF16, tag="x")
            BE = cpool.tile([128, L, KF], F32, tag="be")
            A = cpool.tile([128, L, KF, T], BF16, tag="acts")
            OUT = cpool.tile([128, L, D], BF16, tag="out")

            # ---- PE warm-up: ~32 dummy matmuls during the framework
            # preamble flip the HAM clock gate to 8/8 (2.4 GHz) before the
            # first real matmul; the PE is otherwise idle here.
            WRM = cpool.tile([128, 128], BF16, tag="wrm")
            nc.vector.memset(WRM[:], 0.0)
            wps = pse.tile([128, 128], F32, tag="pse")
            for _ in range(32):
                nc.tensor.matmul(wps[:], WRM[:], WRM[:], start=True, stop=True)

            nc.sync.dma_start(out=BE[:], in_=be_d[:])

            for l in range(L):
                # ---- encode layer l: acts[f, t] = relu(We^T-chunks @ x^T + b)
                nc.sync.dma_start(out=X[:, l, :, :], in_=xt_d[l])
                we = wpool.tile([128, KD, F], BF16, tag="w")
                for kd in range(KD):
                    nc.sync.dma_start(out=we[:, kd, :], in_=we_d[l, kd])
                for ft in range(KF):
                    ps = pse.tile([128, T], F32, tag="pse")
                    for kd in range(KD):
                        nc.tensor.matmul(
                            ps[:],
                            we[:, kd, ts(ft, 128)],
                            X[:, l, kd, :],
                            start=(kd == 0),
                            stop=(kd == KD - 1),
                        )
                    nc.vector.tensor_scalar(
                        out=A[:, l, ft, :],
                        in0=ps[:],
                        scalar1=BE[:, l, ts(ft, 1)],
                        scalar2=0.0,
                        op0=mybir.AluOpType.add,
                        op1=mybir.AluOpType.max,
                    )

                # ---- decode block j=l: recon_j += acts_i^T @ Wq[i,j] (i<=j)
                j = l
                ps0 = psd.tile([128, 384], F32, tag="psd")
                ps1 = psd.tile([128, 384], F32, tag="psd")
                for i in range(j + 1):
                    n = PAIR_IDX[(i, j)]
                    wd = wpool.tile([128, KF, D], BF16, tag="w")
                    if PATH[n] == "S":
                        nc.gpsimd.dma_start(out=wd[:], in_=wq_d[n])
                    else:
                        wq = qpool.tile([128, KF, D], I8, tag="q")
                        nc.sync.dma_start(out=wq[:], in_=wq_d[n])
                        nc.vector.tensor_copy(out=wd[:, 0:3, :], in_=wq[:, 0:3, :])
                        nc.scalar.copy(out=wd[:, 3:6, :], in_=wq[:, 3:6, :])
                    for kf in range(KF):
                        nc.tensor.matmul(
                            ps0[:], A[:, i, kf, :], wd[:, kf, 0:384],
                            start=(i == 0 and kf == 0),
                            stop=(i == j and kf == KF - 1),
                        )
                    for kf in range(KF):
                        nc.tensor.matmul(
                            ps1[:], A[:, i, kf, :], wd[:, kf, 384:768],
                            start=(i == 0 and kf == 0),
                            stop=(i == j and kf == KF - 1),
                        )
                nc.scalar.copy(out=OUT[:, j, 0:384], in_=ps0[:])
                nc.sync.dma_start(out=out_d[j, :, 0:384], in_=OUT[:, j, 0:384])
                nc.vector.tensor_copy(out=OUT[:, j, 384:768], in_=ps1[:])
                nc.sync.dma_start(out=out_d[j, :, 384:768], in_=OUT[:, j, 384:768])

    _split_multiwaits(nc)
    return nc


_NC_CACHE = {}


def _get_nc():
    if "nc" not in _NC_CACHE:
        _NC_CACHE["nc"] = _build_nc()
    return _NC_CACHE["nc"]


def _bf16():
    import ml_dtypes

    return np.dtype(ml_dtypes.bfloat16)


def _prepare(x, W_enc, b_enc, W_dec, dec_mask):
    """Host-side quantization + per-core pre-swizzle into DMA layouts."""
    bf16 = _bf16()
    # per-(source-layer, feature) int8 scale over valid (j >= i) decoders
    if dec_mask is None:
        dec_mask = np.triu(np.ones((L, L), dtype=bool))
    Wd_m = np.where(dec_mask[:, :, None, None], W_dec, 0.0)
    s = np.abs(Wd_m).max(axis=(1, 3)) / 127.0  # [L, DT]
    s = np.where(s == 0, 1.0, s).astype(np.float32)

    # fold the scale into the encoder (relu(s*z) == s*relu(z), s > 0)
    W_enc_f = W_enc * s[:, :, None]
    b_enc_f = b_enc * s

    # xt[l, p, kd, t] = x[l, t, kd*128+p] — same on every core
    xt = np.ascontiguousarray(
        x.transpose(2, 0, 1).reshape(KD, 128, L, T).transpose(2, 1, 0, 3)
    ).astype(bf16)

    in_maps = []
    for c in range(N_CORES):
        fs = c * F
        w = W_enc_f[:, fs : fs + F, :]  # [L, F, D]
        we = np.ascontiguousarray(
            w.transpose(0, 2, 1).reshape(L, KD, 128, F)
        ).astype(bf16)
        be = np.ascontiguousarray(
            b_enc_f[:, fs : fs + F].reshape(L, KF, 128).transpose(2, 0, 1)
        ).astype(np.float32)
        in_maps.append({"xt": xt, "we": we, "be": be})

    # quantize + pack decoder shards: wq[pair, p, kf, d]
    for c in range(N_CORES):
        fs = c * F
        wq = np.empty((NP, 128, KF, D), dtype=np.int8)
        for n, (i, j) in enumerate(PAIRS):
            blk = W_dec[i, j, fs : fs + F, :] / s[i, fs : fs + F, None]
            q = np.rint(blk).clip(-127, 127).astype(np.int8)  # [F, D]
            wq[n] = q.reshape(KF, 128, D).transpose(1, 0, 2)
        in_maps[c]["wq"] = wq
    return in_maps


def kernel(x, W_enc, b_enc, b_dec, W_dec, dec_mask=None, **_unused):
    x = np.asarray(x, dtype=np.float32)
    W_enc = np.asarray(W_enc, dtype=np.float32)
    b_enc = np.asarray(b_enc, dtype=np.float32)
    b_dec = np.asarray(b_dec, dtype=np.float32)
    W_dec = np.asarray(W_dec, dtype=np.float32)

    nc = _get_nc()
    in_maps = _prepare(x, W_enc, b_enc, W_dec, dec_mask)
    res = run_bass_kernel_spmd(nc, in_maps, core_ids=list(range(N_CORES)))

    # host-side all-reduce over feature shards + decoder bias
    recon = np.zeros((L, T, D), dtype=np.float32)
    for c in range(N_CORES):
        recon += res.results[c]["out"].astype(np.float32)
    recon += b_dec[:, None, :]
    return recon
